# revision 7
# baseline (speedup 1.0000x reference)
"""Trainium2 Bass kernel for nn_GPT_26010321945295 (moe_routing).

Sharding: token-parallel over B*T=4096 tokens -> 512 tokens/core.
Core c owns batch b=c//2, half h=c%2 (tokens h*512..h*512+511 of that batch).
Dense MLP and the densely-computed MoE experts need no communication;
attention does a pair AllGather of K/V between the two cores of a batch;
one tiny 8-core AllGather feeds the vocab-sharded lm_head.

Matmuls in bf16 (f32 PSUM accumulation); LN stats, router, gating,
residual stream in f32.
"""
import sys

sys.path.insert(0, "/opt/trn_rl_repo")

import numpy as np
import ml_dtypes

import concourse.bass as bass
import concourse.mybir as mybir
import concourse.tile as tile
from concourse.bass_utils import run_bass_kernel_spmd
from concourse.alu_op_type import AluOpType
from concourse.masks import make_identity

BF16 = mybir.dt.bfloat16
F32 = mybir.dt.float32
AF = mybir.ActivationFunctionType
AX = mybir.AxisListType
NEG = -1e9

D = 1024
NH = 16
HD = 64
HID = 4096
E = 8
V = 50304
T = 1024
B = 4
REC = 2
TPC = 512          # tokens per core
NCORES = 8
VS = V // NCORES   # 6288 vocab shard
SCALE = 1.0 / (HD ** 0.5)

_cache = {}


def _split_multi_waits(nc):
    """Walrus allows 1 sync wait on most instruction structs; hoist extras
    onto same-engine NoOps inserted just before."""
    n = 0
    for block in nc.main_func.blocks:
        out = []
        for inst in block.instructions:
            si = getattr(inst, "sync_info", None)
            waits = list(si.on_wait) if si is not None and si.on_wait else []
            if len(waits) > 1 and not isinstance(inst, mybir.InstNoOp):
                for w in waits[:-1]:
                    out.append(mybir.InstNoOp(
                        name=f"{inst.name}-wn{n}", engine=inst.engine,
                        sync_info=mybir.SyncInfo(on_wait=[w], on_update=[])))
                    n += 1
                inst.sync_info = mybir.SyncInfo(
                    on_wait=waits[-1:], on_update=list(si.on_update))
            out.append(inst)
        block.instructions = out
    return n


def _build(debug=False):
    nc = bass.Bass()
    P = lambda name, shape, dt: nc.declare_dram_parameter(name, list(shape), dt, isOutput=False)

    x0 = P("x0", [TPC, D], F32)
    noise = P("noise", [REC, TPC, E], F32)
    mask = P("mask", [TPC, T], BF16)           # additive causal mask
    wteT = P("wteT", [D, VS], BF16)            # per-core vocab shard of wte.T

    aw0 = P("aw0", [D, 3 * D], BF16); pw0 = P("pw0", [D, D], BF16)
    gw0 = P("gw0", [D, HID], BF16); uw0 = P("uw0", [D, HID], BF16); dw0 = P("dw0", [HID, D], BF16)
    aw1 = P("aw1", [D, 3 * D], BF16); pw1 = P("pw1", [D, D], BF16)
    sgw = P("sgw", [D, HID], BF16); suw = P("suw", [D, HID], BF16); sdw = P("sdw", [HID, D], BF16)
    egw = P("egw", [E, D, HID], BF16); euw = P("euw", [E, D, HID], BF16); edw = P("edw", [E, HID, D], BF16)
    rwnw = P("rwnw", [D, 2 * E], F32)

    logits_part = nc.declare_dram_parameter("logits_part", [NCORES, VS], F32, isOutput=True)
    if debug:
        xtaps = [nc.declare_dram_parameter(f"xtap{i}", [TPC, D], F32, isOutput=True)
                 for i in range(3)]

    with tile.TileContext(nc) as tc:
        import contextlib
        ctx = contextlib.ExitStack()
        with ctx:
            const = ctx.enter_context(tc.tile_pool(name="const", bufs=1))
            sb = ctx.enter_context(tc.tile_pool(name="sb", bufs=2))
            dram = ctx.enter_context(tc.tile_pool(name="dram", bufs=2, space="DRAM"))
            psA = ctx.enter_context(tc.tile_pool(name="psA", bufs=4, space="PSUM"))
            psB = ctx.enter_context(tc.tile_pool(name="psB", bufs=4, space="PSUM"))

            idf = const.tile([128, 128], F32)
            make_identity(nc, idf)
            idb = const.tile([128, 128], BF16)
            make_identity(nc, idb)
            epsT = const.tile([128, 1], F32)
            nc.vector.memset(epsT[:], 1e-5)
            mask_sb = [const.tile([128, T], BF16, name=f"mask{t}") for t in range(4)]
            for t in range(4):
                nc.sync.dma_start(mask_sb[t][:], mask[t * 128:(t + 1) * 128, :])

            NT = TPC // 128   # 4 token tiles
            NF = D // 128     # 8 feature tiles

            def ln_transpose(x_ap, want_f32, tagp=""):
                xnt = [sb.tile([128, TPC], BF16, tag="xnt", name="xnt", bufs=12) for _ in range(NF)]
                xntf = [sb.tile([128, TPC], F32, tag="xntf", name="xntf", bufs=8)
                        for _ in range(NF)] if want_f32 else None
                for t in range(NT):
                    xt = sb.tile([128, D], F32, tag="ln_x")
                    nc.sync.dma_start(xt[:], x_ap[t * 128:(t + 1) * 128, :])
                    mean = sb.tile([128, 1], F32, tag="ln_m")
                    nc.vector.reduce_sum(mean[:], xt[:], AX.X)
                    nc.scalar.mul(mean[:], mean[:], 1.0 / D)
                    xc = sb.tile([128, D], F32, tag="ln_xc")
                    nc.vector.tensor_scalar(out=xc[:], in0=xt[:], scalar1=mean[:],
                                            scalar2=None, op0=AluOpType.subtract)
                    sqf = sb.tile([128, D], F32, tag="ln_sq", bufs=1)
                    var = sb.tile([128, 1], F32, tag="ln_v")
                    nc.scalar.activation(sqf[:], xc[:], AF.Square, accum_out=var[:])
                    std = sb.tile([128, 1], F32, tag="ln_s")
                    nc.scalar.activation(std[:], var[:], AF.Sqrt, bias=epsT[:], scale=1.0 / D)
                    rstd = sb.tile([128, 1], F32, tag="ln_r")
                    nc.vector.reciprocal(rstd[:], std[:])
                    xn = sb.tile([128, D], F32, tag="ln_xn")
                    nc.vector.tensor_scalar(out=xn[:], in0=xc[:], scalar1=rstd[:],
                                            scalar2=None, op0=AluOpType.mult)
                    for f in range(NF):
                        pt = psA.tile([128, 512], F32, tag="b512")
                        nc.tensor.transpose(pt[:, :128], xn[:, f * 128:(f + 1) * 128], idf[:])
                        cols = slice(t * 128, (t + 1) * 128)
                        nc.any.tensor_copy(out=xnt[f][:, cols], in_=pt[:, :128])
                        if want_f32:
                            nc.any.tensor_copy(out=xntf[f][:, cols], in_=pt[:, :128])
                return xnt, xntf

            def swiglu_into(xnt, gw_ap, uw_ap, dw_ap, out_acc, gate=None):
                """out_acc[t] += (silu(xn@gw) * (xn@uw)) @ dw [* gate], via two
                half-hidden passes; down accumulates in 8 psum banks... (4 tiles x2 cols)"""
                NH_T = HID // 128  # 32 hidden tiles
                for half in range(2):
                    prod = sb.tile([128, NH_T // 2, TPC], BF16, tag="prod", name="prod", bufs=1)
                    for htq in range(8):          # 2 hidden tiles per fetch
                        hbase = half * 16 + htq * 2
                        gpan = sb.tile([128, NF, 256], BF16, tag="gupan", bufs=3)
                        upan = sb.tile([128, NF, 256], BF16, tag="gupan", bufs=3)
                        gsrc = gw_ap[:, hbase * 128:(hbase + 2) * 128]
                        usrc = uw_ap[:, hbase * 128:(hbase + 2) * 128]
                        nc.sync.dma_start(gpan[:], gsrc.rearrange("(f p) m -> p f m", p=128))
                        nc.sync.dma_start(upan[:], usrc.rearrange("(f p) m -> p f m", p=128))
                        for hi in range(2):
                            ht = htq * 2 + hi
                            pg = psA.tile([128, 512], F32, tag="b512")
                            pu = psA.tile([128, 512], F32, tag="b512")
                            for k in range(NF):
                                nc.tensor.matmul(pg[:], gpan[:, k, hi * 128:(hi + 1) * 128],
                                                 xnt[k][:], start=(k == 0), stop=(k == NF - 1))
                            for k in range(NF):
                                nc.tensor.matmul(pu[:], upan[:, k, hi * 128:(hi + 1) * 128],
                                                 xnt[k][:], start=(k == 0), stop=(k == NF - 1))
                            sil = sb.tile([128, TPC], BF16, tag="sil")
                            nc.scalar.activation(sil[:], pg[:], AF.Silu)
                            nc.vector.tensor_tensor(out=prod[:, ht, :], in0=sil[:], in1=pu[:],
                                                    op=AluOpType.mult)
                    for no in range(2):
                        pd = [psB.tile([128, 512], F32, tag="b512d", name="pd")
                              for _ in range(NT)]
                        for kt in range(NH_T // 2):
                            gkt = half * 16 + kt
                            dpan = sb.tile([128, 512], BF16, tag="dpan", bufs=3)
                            nc.sync.dma_start(dpan[:], dw_ap[gkt * 128:(gkt + 1) * 128,
                                                             no * 512:(no + 1) * 512])
                            for mt in range(NT):
                                nc.tensor.matmul(pd[mt][:],
                                                 prod[:, kt, mt * 128:(mt + 1) * 128],
                                                 dpan[:],
                                                 start=(kt == 0),
                                                 stop=(kt == NH_T // 2 - 1))
                        for mt in range(NT):
                            cols = slice(no * 512, (no + 1) * 512)
                            if gate is not None:
                                nc.vector.tensor_scalar(out=pd[mt][:], in0=pd[mt][:],
                                                        scalar1=gate[mt], scalar2=None,
                                                        op0=AluOpType.mult)
                            nc.vector.tensor_tensor(out=out_acc[mt][:, cols],
                                                    in0=out_acc[mt][:, cols], in1=pd[mt][:],
                                                    op=AluOpType.add)

            def attn_stage(x_ap, aw_ap, pw_ap, x_out):
                xnt, _ = ln_transpose(x_ap, False)
                qkvT = dram.tile([3 * D, TPC], BF16, tag="qkvT", name="qkvT")
                for m in range(3 * D // 128):
                    wpan = sb.tile([128, NF, 128], BF16, tag="wpan2k", bufs=3, name="awpan")
                    src = aw_ap[:, m * 128:(m + 1) * 128]
                    nc.sync.dma_start(wpan[:], src.rearrange("(f p) m -> p f m", p=128))
                    pq = psA.tile([128, 512], F32, tag="b512")
                    for k in range(NF):
                        nc.tensor.matmul(pq[:], wpan[:, k, :], xnt[k][:],
                                         start=(k == 0), stop=(k == NF - 1))
                    ev = sb.tile([128, TPC], BF16, tag="qkv_ev")
                    s = SCALE if m < NF else 1.0
                    nc.scalar.activation(ev[:], pq[:], AF.Copy, scale=s)
                    nc.sync.dma_start(qkvT[m * 128:(m + 1) * 128, :], ev[:])
                kv_in = dram.tile([2 * D, TPC], BF16, tag="kv_in")
                nc.sync.dma_start(kv_in[:], qkvT[D:3 * D, :])
                kv_g = dram.tile([2, 2 * D, TPC], BF16, tag="kv_g")
                nc.gpsimd.collective_compute(
                    "AllGather", AluOpType.bypass,
                    replica_groups=[[0, 1], [2, 3], [4, 5], [6, 7]],
                    ins=[kv_in[:]], outs=[kv_g[:]])
                kT = dram.tile([D, T], BF16, tag="kT")
                for r in range(2):
                    nc.sync.dma_start(kT[:, r * TPC:(r + 1) * TPC], kv_g[r, 0:D, :])
                vtok = dram.tile([T, D], BF16, tag="vtok")
                for r in range(2):
                    for th in range(2):
                        evs = [sb.tile([128, D], BF16, tag="vt_ev", name="vt_ev", bufs=2)
                               for _ in range(2)]
                        for f in range(NF):
                            vl = sb.tile([128, TPC], BF16, tag="vt_ld")
                            nc.sync.dma_start(vl[:], kv_g[r, D + f * 128:D + (f + 1) * 128, :])
                            for ti in range(2):
                                t = th * 2 + ti
                                pt = psA.tile([128, 512], BF16, tag="b512")
                                nc.tensor.transpose(pt[:, :128], vl[:, t * 128:(t + 1) * 128], idb[:])
                                nc.any.tensor_copy(out=evs[ti][:, f * 128:(f + 1) * 128],
                                                   in_=pt[:, :128])
                        for ti in range(2):
                            row = r * TPC + (th * 2 + ti) * 128
                            nc.sync.dma_start(vtok[row:row + 128, :], evs[ti][:])
                yT = dram.tile([D, TPC], BF16, tag="yT")
                for hp in range(NH // 2):
                    qp = sb.tile([128, TPC], BF16, tag="qp")
                    nc.sync.dma_start(qp[:], qkvT[hp * 128:(hp + 1) * 128, :])
                    kp = sb.tile([128, T], BF16, tag="kp")
                    nc.sync.dma_start(kp[:], kT[hp * 128:(hp + 1) * 128, :])
                    for hh in range(2):
                        h0 = hh * 64
                        probsT = [sb.tile([128, TPC], BF16, tag="probsT", name="probsT", bufs=8)
                                  for _ in range(T // 128)]
                        for mt in range(NT):
                            s_sb = sb.tile([128, T], F32, tag="s_sb")
                            for nt2 in range(2):
                                ps = psA.tile([128, 512], F32, tag="b512")
                                nc.tensor.matmul(ps[:], qp[h0:h0 + 64, mt * 128:(mt + 1) * 128],
                                                 kp[h0:h0 + 64, nt2 * 512:(nt2 + 1) * 512],
                                                 start=True, stop=True)
                                cols = slice(nt2 * 512, (nt2 + 1) * 512)
                                nc.vector.tensor_tensor(out=s_sb[:, cols], in0=ps[:],
                                                        in1=mask_sb[mt][:, cols], op=AluOpType.add)
                            nmax = sb.tile([128, 1], F32, tag="nmax")
                            nc.vector.reduce_max(nmax[:], s_sb[:], AX.X, negate=True)
                            probs = sb.tile([128, T], BF16, tag="probs")
                            rsum = sb.tile([128, 1], F32, tag="rsum")
                            nc.scalar.activation(probs[:], s_sb[:], AF.Exp, bias=nmax[:],
                                                 accum_out=rsum[:])
                            rrec = sb.tile([128, 1], F32, tag="rrec")
                            nc.vector.reciprocal(rrec[:], rsum[:])
                            nc.vector.tensor_scalar(out=probs[:], in0=probs[:], scalar1=rrec[:],
                                                    scalar2=None, op0=AluOpType.mult)
                            for f in range(T // 128):
                                pt = psA.tile([128, 512], BF16, tag="b512")
                                nc.tensor.transpose(pt[:, :128], probs[:, f * 128:(f + 1) * 128], idb[:])
                                nc.any.tensor_copy(out=probsT[f][:, mt * 128:(mt + 1) * 128],
                                                   in_=pt[:, :128])
                        pav = psB.tile([128, 512], F32, tag="b512d")
                        for kt in range(T // 128):
                            vls = sb.tile([128, 64], BF16, tag="av_v")
                            nc.sync.dma_start(vls[:], vtok[kt * 128:(kt + 1) * 128,
                                                           hp * 128 + h0:hp * 128 + h0 + 64])
                            nc.tensor.matmul(pav[:64, :], vls[:], probsT[kt][:],
                                             start=(kt == 0), stop=(kt == T // 128 - 1))
                        yev = sb.tile([128, TPC], BF16, tag="yev", name="yev")
                        nc.any.tensor_copy(out=yev[:64, :], in_=pav[:64, :])
                        nc.sync.dma_start(yT[hp * 128 + h0:hp * 128 + h0 + 64, :], yev[:64, :])
                for mt in range(NT):
                    po = [psB.tile([128, 512], F32, tag="b512d", name="po") for _ in range(2)]
                    for kt in range(NF):
                        ylhs = sb.tile([128, 128], BF16, tag="ylhs")
                        nc.sync.dma_start(ylhs[:], yT[kt * 128:(kt + 1) * 128,
                                                      mt * 128:(mt + 1) * 128])
                        ppan = sb.tile([128, D], BF16, tag="wpan2k", bufs=3, name="ppan")
                        nc.sync.dma_start(ppan[:], pw_ap[kt * 128:(kt + 1) * 128, :])
                        for no in range(2):
                            nc.tensor.matmul(po[no][:], ylhs[:], ppan[:, no * 512:(no + 1) * 512],
                                             start=(kt == 0), stop=(kt == NF - 1))
                    xt = sb.tile([128, D], F32, tag="res_x")
                    nc.sync.dma_start(xt[:], x_ap[mt * 128:(mt + 1) * 128, :])
                    for no in range(2):
                        cols = slice(no * 512, (no + 1) * 512)
                        nc.vector.tensor_tensor(out=xt[:, cols], in0=xt[:, cols],
                                                in1=po[no][:], op=AluOpType.add)
                    nc.sync.dma_start(x_out[mt * 128:(mt + 1) * 128, :], xt[:])

            def mlp_stage(x_ap, x_out, gw_ap, uw_ap, dw_ap):
                xnt, _ = ln_transpose(x_ap, False)
                out_acc = [sb.tile([128, D], F32, tag="oacc", name="oacc", bufs=4)
                           for _ in range(NT)]
                for mt in range(NT):
                    nc.sync.dma_start(out_acc[mt][:], x_ap[mt * 128:(mt + 1) * 128, :])
                swiglu_into(xnt, gw_ap, uw_ap, dw_ap, out_acc, None)
                for mt in range(NT):
                    nc.sync.dma_start(x_out[mt * 128:(mt + 1) * 128, :], out_acc[mt][:])

            def moe_stage(x_ap, x_out, rec):
                xnt, xntf = ln_transpose(x_ap, True)
                rw_sb = sb.tile([128, NF, 2 * E], F32, tag="rw")
                nc.sync.dma_start(rw_sb[:], rwnw.rearrange("(f p) m -> p f m", p=128))
                pr = psA.tile([128, 512], F32, tag="b512")
                for k in range(NF):
                    nc.tensor.matmul(pr[:16, :], rw_sb[:, k, :], xntf[k][:],
                                     start=(k == 0), stop=(k == NF - 1))
                r_sb = sb.tile([128, TPC], F32, tag="r_sb")
                nc.any.tensor_copy(out=r_sb[:16, :], in_=pr[:16, :])
                gates = []
                for mt in range(NT):
                    pt = psA.tile([128, 512], F32, tag="b512")
                    nc.tensor.transpose(pt[:, :16], r_sb[:16, mt * 128:(mt + 1) * 128],
                                        idf[:16, :16])
                    lg = sb.tile([128, 2 * E], F32, tag="lg")
                    nc.any.tensor_copy(out=lg[:], in_=pt[:, :16])
                    nz = sb.tile([128, E], F32, tag="nz")
                    nc.sync.dma_start(nz[:], noise[rec, mt * 128:(mt + 1) * 128, :])
                    esp = sb.tile([128, E], F32, tag="esp")
                    nc.scalar.activation(esp[:], lg[:, E:2 * E], AF.Exp)
                    sp = sb.tile([128, E], F32, tag="sp")
                    nc.scalar.activation(sp[:], esp[:], AF.Ln, bias=1.0)
                    nsy = sb.tile([128, E], F32, tag="nsy")
                    nc.vector.tensor_tensor(out=nsy[:], in0=nz[:], in1=sp[:], op=AluOpType.mult)
                    nc.vector.tensor_tensor(out=nsy[:], in0=nsy[:], in1=lg[:, 0:E], op=AluOpType.add)
                    nm1 = sb.tile([128, 1], F32, tag="nm1")
                    nc.vector.reduce_max(nm1[:], nsy[:], AX.X, negate=True)
                    m1 = sb.tile([128, 1], F32, tag="m1")
                    nc.vector.reduce_max(m1[:], nsy[:], AX.X)
                    ismax = sb.tile([128, E], F32, tag="ismax")
                    nc.vector.tensor_scalar(out=ismax[:], in0=nsy[:], scalar1=m1[:],
                                            scalar2=None, op0=AluOpType.is_ge)
                    big = sb.tile([128, E], F32, tag="big")
                    nc.scalar.mul(big[:], ismax[:], 1e9)
                    msk2 = sb.tile([128, E], F32, tag="msk2")
                    nc.vector.tensor_tensor(out=msk2[:], in0=nsy[:], in1=big[:],
                                            op=AluOpType.subtract)
                    m2 = sb.tile([128, 1], F32, tag="m2")
                    nc.vector.reduce_max(m2[:], msk2[:], AX.X)
                    keep = sb.tile([128, E], F32, tag="keep")
                    nc.vector.tensor_scalar(out=keep[:], in0=nsy[:], scalar1=m2[:],
                                            scalar2=None, op0=AluOpType.is_ge)
                    ex = sb.tile([128, E], F32, tag="ex")
                    nc.scalar.activation(ex[:], nsy[:], AF.Exp, bias=nm1[:])
                    gex = sb.tile([128, E], F32, tag="gex")
                    nc.vector.tensor_tensor(out=gex[:], in0=ex[:], in1=keep[:], op=AluOpType.mult)
                    gsum = sb.tile([128, 1], F32, tag="gsum")
                    nc.vector.reduce_sum(gsum[:], gex[:], AX.X)
                    grec = sb.tile([128, 1], F32, tag="grec")
                    nc.vector.reciprocal(grec[:], gsum[:])
                    gt = sb.tile([128, E], F32, tag="gate", name="gate", bufs=8)
                    nc.vector.tensor_scalar(out=gt[:], in0=gex[:], scalar1=grec[:],
                                            scalar2=None, op0=AluOpType.mult)
                    gates.append(gt)
                out_acc = [sb.tile([128, D], F32, tag="oacc", name="oacc", bufs=4)
                           for _ in range(NT)]
                for mt in range(NT):
                    nc.sync.dma_start(out_acc[mt][:], x_ap[mt * 128:(mt + 1) * 128, :])
                swiglu_into(xnt, sgw, suw, sdw, out_acc, None)
                for e in range(E):
                    swiglu_into(xnt, egw[e], euw[e], edw[e], out_acc,
                                [gates[mt][:, e:e + 1] for mt in range(NT)])
                for mt in range(NT):
                    nc.sync.dma_start(x_out[mt * 128:(mt + 1) * 128, :], out_acc[mt][:])

            # ---- forward ----
            xbufs = [dram.tile([TPC, D], F32, tag="xb", name=f"xb{i}", bufs=4) for i in range(4)]
            attn_stage(x0, aw0, pw0, xbufs[0])
            mlp_stage(xbufs[0], xbufs[1], gw0, uw0, dw0)
            attn_stage(xbufs[1], aw0, pw0, xbufs[2])
            mlp_stage(xbufs[2], xbufs[3], gw0, uw0, dw0)
            if debug:
                nc.sync.dma_start(xtaps[0][:], xbufs[3][:])
            xb2 = [dram.tile([TPC, D], F32, tag="xb2", name=f"xc{i}", bufs=4) for i in range(4)]
            attn_stage(xbufs[3], aw1, pw1, xb2[0])
            moe_stage(xb2[0], xb2[1], 0)
            if debug:
                nc.sync.dma_start(xtaps[1][:], xb2[1][:])
            attn_stage(xb2[1], aw1, pw1, xb2[2])
            moe_stage(xb2[2], xb2[3], 1)
            if debug:
                nc.sync.dma_start(xtaps[2][:], xb2[3][:])

            # ---- final: gather last tokens, lnf, lm_head ----
            xl_in = dram.tile([1, D], F32, tag="xl_in")
            nc.sync.dma_start(xl_in[:], xb2[3][TPC - 1:TPC, :])
            xl_g = dram.tile([NCORES, D], F32, tag="xl_g")
            nc.gpsimd.collective_compute(
                "AllGather", AluOpType.bypass,
                replica_groups=[[0, 1, 2, 3, 4, 5, 6, 7]],
                ins=[xl_in[:]], outs=[xl_g[:]])
            xf = sb.tile([128, D], F32, tag="ln_x", name="xf")
            nc.sync.dma_start(xf[:NCORES, :], xl_g[:])
            mean = sb.tile([128, 1], F32, tag="ln_m", name="fmean")
            nc.vector.reduce_sum(mean[:NCORES], xf[:NCORES, :], AX.X)
            nc.scalar.mul(mean[:NCORES], mean[:NCORES], 1.0 / D)
            xc = sb.tile([128, D], F32, tag="ln_xc", name="fxc")
            nc.vector.tensor_scalar(out=xc[:NCORES], in0=xf[:NCORES], scalar1=mean[:NCORES],
                                    scalar2=None, op0=AluOpType.subtract)
            sqf = sb.tile([128, D], F32, tag="ln_sq", bufs=1, name="fsq")
            var = sb.tile([128, 1], F32, tag="ln_v", name="fvar")
            nc.scalar.activation(sqf[:NCORES], xc[:NCORES], AF.Square, accum_out=var[:NCORES])
            std = sb.tile([128, 1], F32, tag="ln_s", name="fstd")
            nc.scalar.activation(std[:NCORES], var[:NCORES], AF.Sqrt, bias=epsT[:NCORES],
                                 scale=1.0 / D)
            rstd = sb.tile([128, 1], F32, tag="ln_r", name="frstd")
            nc.vector.reciprocal(rstd[:NCORES], std[:NCORES])
            xn = sb.tile([128, D], F32, tag="ln_xn", name="fxn")
            nc.vector.tensor_scalar(out=xn[:NCORES], in0=xc[:NCORES], scalar1=rstd[:NCORES],
                                    scalar2=None, op0=AluOpType.mult)
            xfT = sb.tile([128, NF, NCORES], BF16, tag="xfT")
            for f in range(NF):
                pt = psA.tile([128, 512], F32, tag="b512")
                nc.tensor.transpose(pt[:, :NCORES], xn[:NCORES, f * 128:(f + 1) * 128],
                                    idf[:NCORES, :NCORES])
                nc.any.tensor_copy(out=xfT[:, f, :], in_=pt[:, :NCORES])
            nchunks = (VS + 511) // 512
            for ci in range(nchunks):
                w = min(512, VS - ci * 512)
                pl = psB.tile([128, 512], F32, tag="b512d", name="pl")
                for k in range(NF):
                    wpan = sb.tile([128, 512], BF16, tag="lm_w")
                    nc.sync.dma_start(wpan[:, :w], wteT[k * 128:(k + 1) * 128,
                                                       ci * 512:ci * 512 + w])
                    nc.tensor.matmul(pl[:NCORES, :w], xfT[:, k, :], wpan[:, :w],
                                     start=(k == 0), stop=(k == NF - 1))
                lev = sb.tile([128, 512], F32, tag="lm_ev")
                nc.any.tensor_copy(out=lev[:NCORES, :w], in_=pl[:NCORES, :w])
                nc.sync.dma_start(logits_part[:, ci * 512:ci * 512 + w], lev[:NCORES, :w])

    _split_multi_waits(nc)
    return nc


def _prep_host(params, idx, router_noise):
    bf = ml_dtypes.bfloat16
    wte = np.asarray(params["wte"], np.float32)
    wpe = np.asarray(params["wpe"], np.float32)
    idx = np.asarray(idx)
    x_full = wte[idx] + wpe[None, :, :]        # [B, T, D] f32

    def t2(w):  # [O, I] -> [I, O] bf16
        return np.ascontiguousarray(np.asarray(w, np.float32).T).astype(bf)

    b0, b1 = params["b0"], params["b1"]
    moe = b1["moe"]
    shared = {
        "aw0": t2(b0["attn"]["aw"]), "pw0": t2(b0["attn"]["pw"]),
        "gw0": t2(b0["mlp"]["gw"]), "uw0": t2(b0["mlp"]["uw"]), "dw0": t2(b0["mlp"]["dw"]),
        "aw1": t2(b1["attn"]["aw"]), "pw1": t2(b1["attn"]["pw"]),
        "sgw": t2(moe["shared"]["gw"]), "suw": t2(moe["shared"]["uw"]), "sdw": t2(moe["shared"]["dw"]),
        "egw": np.ascontiguousarray(np.asarray(moe["egw"], np.float32).transpose(0, 2, 1)).astype(bf),
        "euw": np.ascontiguousarray(np.asarray(moe["euw"], np.float32).transpose(0, 2, 1)).astype(bf),
        "edw": np.ascontiguousarray(np.asarray(moe["edw"], np.float32).transpose(0, 2, 1)).astype(bf),
        "rwnw": np.ascontiguousarray(
            np.concatenate([np.asarray(moe["rw"], np.float32),
                            np.asarray(moe["nw"], np.float32)], 0).T),
    }
    wteT = np.ascontiguousarray(wte.T).astype(bf)      # [D, V]
    noise = np.asarray(router_noise, np.float32)       # [REC, B, T, E]

    masks = []
    for h in range(2):
        gq = (h * TPC + np.arange(TPC))[:, None]
        kj = np.arange(T)[None, :]
        masks.append(np.where(kj <= gq, 0.0, NEG).astype(bf))

    in_maps = []
    for c in range(NCORES):
        b, h = c // 2, c % 2
        sl = slice(h * TPC, (h + 1) * TPC)
        m = dict(shared)
        m["x0"] = np.ascontiguousarray(x_full[b, sl])
        m["noise"] = np.ascontiguousarray(noise[:, b, sl, :])
        m["mask"] = masks[h]
        m["wteT"] = np.ascontiguousarray(wteT[:, c * VS:(c + 1) * VS])
        in_maps.append(m)

    ln_identity = (
        np.allclose(np.asarray(b0["ln1w"]), 1) and np.allclose(np.asarray(b0["ln1b"]), 0)
        and np.allclose(np.asarray(b0["ln2w"]), 1) and np.allclose(np.asarray(b0["ln2b"]), 0)
        and np.allclose(np.asarray(b1["ln1w"]), 1) and np.allclose(np.asarray(b1["ln1b"]), 0)
        and np.allclose(np.asarray(b1["ln2w"]), 1) and np.allclose(np.asarray(b1["ln2b"]), 0)
        and np.allclose(np.asarray(params["lnfw"]), 1) and np.allclose(np.asarray(params["lnfb"]), 0))
    zero_bias = all(
        not np.any(np.asarray(z))
        for z in [b0["attn"]["ab"], b0["attn"]["pb"], b0["mlp"]["gb"], b0["mlp"]["ub"],
                  b0["mlp"]["db"], b1["attn"]["ab"], b1["attn"]["pb"],
                  moe["rb"], moe["nb"], moe["shared"]["gb"], moe["shared"]["ub"],
                  moe["shared"]["db"], moe["egb"], moe["eub"], moe["edb"]])
    assert ln_identity and zero_bias, "kernel compiled for identity LN affine / zero biases"
    return in_maps


def run(params, idx, router_noise, debug=False, trace=False):
    key = ("dbg" if debug else "perf")
    if key not in _cache:
        _cache[key] = _build(debug=debug)
    nc = _cache[key]
    in_maps = _prep_host(params, idx, router_noise)
    res = run_bass_kernel_spmd(nc, in_maps, list(range(NCORES)), trace=trace)
    out = np.empty((B, 1, V), np.float32)
    for b in range(B):
        row = 2 * b + 1
        for c in range(NCORES):
            out[b, 0, c * VS:(c + 1) * VS] = res.results[c]["logits_part"][row]
    return out, res


def kernel(params, idx, router_noise):
    out, _ = run(params, idx, router_noise)
    return out
